# revision 1
# baseline (speedup 1.0000x reference)
"""Trainium2 Bass kernel for nn_DecoderOutputLayer (scatter_memory).

Sharding across 8 NeuronCores:
  - memory/copy branch: data-parallel over batch (4 rows per core). Each core
    computes attention scores w = q.k over its rows' 2048 memory slots, the
    softmax numerators e = exp(w - max), and the unnormalized weighted memory
    sum vbar = sum_j e_j * memencs_j.
  - generation branch: tensor-parallel over vocab (6283 rows of W_out per
    core). Each core computes u = exp(h @ W_shard.T + bias) for all 32 batch
    rows plus its partial softmax denominator.
Host side: shards/lays out inputs, then combines shard outputs (softmax
normalizations, mixer weighting, scatter-add of the 65K attention weights
into the vocab axis).
"""

from contextlib import ExitStack

import numpy as np

import concourse.bacc as bacc
import concourse.bass_isa as bass_isa
import concourse.mybir as mybir
import concourse.tile as tile
from concourse.bass_utils import run_bass_kernel_spmd

F32 = mybir.dt.float32
AX = mybir.AxisListType
ALU = mybir.AluOpType
ACTF = mybir.ActivationFunctionType

# Problem shapes (hardcoded per contest contract)
B, DIM, EMBDIM = 32, 512, 512
M, Z = 32, 64
VOCAB = 50257
NCORES = 8
B_LOC = B // NCORES        # 4 batch rows per core
J = M * Z                  # 2048 memory slots per row
NJT = J // 128             # 16 j-tiles of 128
KDIM = 2 * DIM             # 1024 gen contraction dim
VSH = 6283                 # vocab shard per core (8*6283 = 50264 >= 50257)
VSLAB = 256                # vocab columns per gen matmul slab
NB = B                     # full batch in gen branch


def _build(nc):
    mem_enc = nc.dram_tensor("mem_enc", [B_LOC, NJT, 128, DIM], F32, kind="ExternalInput")
    mem_emb = nc.dram_tensor("mem_emb", [B_LOC, NJT, 128, DIM], F32, kind="ExternalInput")
    q_d = nc.dram_tensor("q", [1, B_LOC * KDIM], F32, kind="ExternalInput")
    logmask_d = nc.dram_tensor("logmask", [128, B_LOC * NJT], F32, kind="ExternalInput")
    wt_d = nc.dram_tensor("wt", [KDIM, VSH], F32, kind="ExternalInput")
    ht_d = nc.dram_tensor("ht", [KDIM, NB], F32, kind="ExternalInput")
    bias_d = nc.dram_tensor("bias_gen", [1, VSH], F32, kind="ExternalInput")

    e_out = nc.dram_tensor("e_out", [128, B_LOC * NJT], F32, kind="ExternalOutput")
    u_out = nc.dram_tensor("u_out", [NB, VSH], F32, kind="ExternalOutput")
    sg_out = nc.dram_tensor("sg_out", [NB, 1], F32, kind="ExternalOutput")
    vb_out = nc.dram_tensor("vb_out", [1, B_LOC * DIM], F32, kind="ExternalOutput")

    nslab = (VSH + VSLAB - 1) // VSLAB

    with tile.TileContext(nc) as tc:
        with ExitStack() as ctx:
            consts = ctx.enter_context(tc.tile_pool(name="consts", bufs=1))
            resident = ctx.enter_context(tc.tile_pool(name="resident", bufs=1))
            stream = ctx.enter_context(tc.tile_pool(name="stream", bufs=2))
            scratch = ctx.enter_context(tc.tile_pool(name="scratch", bufs=4))
            small = ctx.enter_context(tc.tile_pool(name="small", bufs=1))
            upool = ctx.enter_context(tc.tile_pool(name="upool", bufs=3))
            psum = ctx.enter_context(tc.tile_pool(name="psum", bufs=2, space="PSUM"))
            psum_vb = ctx.enter_context(tc.tile_pool(name="psum_vb", bufs=1, space="PSUM"))
            psum_gen = ctx.enter_context(tc.tile_pool(name="psum_gen", bufs=2, space="PSUM"))

            pos_ones = consts.tile([1, 128], F32)
            nc.vector.memset(pos_ones[:], 1.0)

            logmask_t = small.tile([128, B_LOC * NJT], F32, tag="logmask")
            nc.sync.dma_start(out=logmask_t[:], in_=logmask_d[:])
            ht_t = small.tile([128, 8 * NB], F32, tag="ht")
            nc.sync.dma_start(
                out=ht_t[:].rearrange("p (c b) -> p c b", b=NB),
                in_=ht_d.ap().rearrange("(c p) b -> p c b", p=128),
            )

            # replicate per-row queries to all 128 partitions via PE broadcast
            q_rep = small.tile([128, B_LOC * KDIM], F32, tag="q_rep")
            for r in range(B_LOC):
                for half in range(2):
                    qtmp = scratch.tile([1, DIM], F32, tag="qtmp")
                    nc.sync.dma_start(
                        out=qtmp[:],
                        in_=q_d[:, (r * 2 + half) * DIM : (r * 2 + half + 1) * DIM],
                    )
                    bp = psum.tile([128, DIM], F32, tag="bcast", space="PSUM")
                    nc.tensor.matmul(
                        out=bp[:], lhsT=pos_ones[:], rhs=qtmp[:], start=True, stop=True
                    )
                    nc.scalar.copy(
                        out=q_rep[:, (r * 2 + half) * DIM : (r * 2 + half + 1) * DIM],
                        in_=bp[:],
                    )

            mem_res = resident.tile([128, B_LOC * NJT * DIM], F32, tag="mem_res")

            # attention scores: w = q_enc.mem_enc + q_emb.mem_emb + logmask
            wenc_t = small.tile([128, B_LOC * NJT], F32, tag="wenc")
            wemb_t = small.tile([128, B_LOC * NJT], F32, tag="wemb")
            w_t = small.tile([128, B_LOC * NJT], F32, tag="w")
            e_t = small.tile([128, B_LOC * NJT], F32, tag="e")

            for r in range(B_LOC):
                for jt in range(NJT):
                    f = r * NJT + jt
                    rslice = slice(f * DIM, (f + 1) * DIM)
                    nc.sync.dma_start(out=mem_res[:, rslice], in_=mem_enc[r, jt])
                    sc1 = scratch.tile([128, DIM], F32, tag="ttr_sc")
                    nc.vector.tensor_tensor(
                        out=sc1[:],
                        in0=mem_res[:, rslice],
                        in1=q_rep[:, (r * 2) * DIM : (r * 2 + 1) * DIM],
                        op=ALU.mult,
                    )
                    nc.scalar.activation(
                        out=sc1[:], in_=sc1[:], func=ACTF.Copy,
                        accum_out=wenc_t[:, f : f + 1],
                    )
                    emb_t = stream.tile([128, DIM], F32, tag="emb")
                    nc.sync.dma_start(out=emb_t[:], in_=mem_emb[r, jt])
                    sc2 = scratch.tile([128, DIM], F32, tag="ttr_sc")
                    nc.vector.tensor_tensor(
                        out=sc2[:],
                        in0=emb_t[:],
                        in1=q_rep[:, (r * 2 + 1) * DIM : (r * 2 + 2) * DIM],
                        op=ALU.mult,
                    )
                    nc.scalar.activation(
                        out=sc2[:], in_=sc2[:], func=ACTF.Copy,
                        accum_out=wemb_t[:, f : f + 1],
                    )
            nc.vector.tensor_tensor(out=w_t[:], in0=wenc_t[:], in1=wemb_t[:], op=ALU.add)
            nc.vector.tensor_tensor(out=w_t[:], in0=w_t[:], in1=logmask_t[:], op=ALU.add)

            # per-row softmax numerators e = exp(w - max_r)
            mx_p = small.tile([128, B_LOC], F32, tag="mx_p")
            allm = small.tile([128, B_LOC], F32, tag="allm")
            negm = small.tile([128, B_LOC], F32, tag="negm_sb")
            for r in range(B_LOC):
                rcols = slice(r * NJT, (r + 1) * NJT)
                nc.vector.tensor_reduce(
                    out=mx_p[:, r : r + 1], in_=w_t[:, rcols], axis=AX.X, op=ALU.max
                )
                nc.gpsimd.partition_all_reduce(
                    out_ap=allm[:, r : r + 1],
                    in_ap=mx_p[:, r : r + 1],
                    channels=128,
                    reduce_op=bass_isa.ReduceOp.max,
                )
                nc.vector.tensor_scalar_mul(
                    out=negm[:, r : r + 1], in0=allm[:, r : r + 1], scalar1=-1.0
                )
                nc.scalar.activation(
                    out=e_t[:, rcols], in_=w_t[:, rcols], func=ACTF.Exp,
                    bias=negm[:, r : r + 1],
                )
            nc.sync.dma_start(out=e_out[:], in_=e_t[:])

            # vbar[r] = sum_j e[r, j] * mem_enc[r, j, :]
            vb_ps = psum_vb.tile([1, B_LOC * DIM], F32, tag="vb", space="PSUM")
            for r in range(B_LOC):
                for jt in range(NJT):
                    f = r * NJT + jt
                    nc.tensor.matmul(
                        out=vb_ps[:, r * DIM : (r + 1) * DIM],
                        lhsT=e_t[:, f : f + 1],
                        rhs=mem_res[:, f * DIM : (f + 1) * DIM],
                        start=(jt == 0),
                        stop=(jt == NJT - 1),
                    )
            vb_sb = small.tile([1, B_LOC * DIM], F32, tag="vb_sb")
            nc.scalar.copy(out=vb_sb[:], in_=vb_ps[:])
            nc.sync.dma_start(out=vb_out[:], in_=vb_sb[:])

            # gen branch: u = exp(h @ W_shard.T + bias), sg = shard row sums
            sg_cols = small.tile([NB, nslab], F32, tag="sg_cols")
            wt_r = wt_d.ap().rearrange("(c p) v -> p c v", p=128)
            for s in range(nslab):
                v0 = s * VSLAB
                v1 = min(v0 + VSLAB, VSH)
                vn = v1 - v0
                w_sl = stream.tile([128, 8 * VSLAB], F32, tag="w_slab")
                nc.sync.dma_start(
                    out=w_sl[:].rearrange("p (c v) -> p c v", v=VSLAB)[:, :, :vn],
                    in_=wt_r[:, :, v0:v1],
                )
                lg_ps = psum_gen.tile([NB, VSLAB], F32, tag="lg", space="PSUM")
                for kc in range(8):
                    nc.tensor.matmul(
                        out=lg_ps[:, :vn],
                        lhsT=ht_t[:, kc * NB : (kc + 1) * NB],
                        rhs=w_sl[:, kc * VSLAB : kc * VSLAB + vn],
                        start=(kc == 0),
                        stop=False,
                    )
                bias_sl = scratch.tile([1, VSLAB], F32, tag="bias_sl")
                nc.sync.dma_start(out=bias_sl[:, :vn], in_=bias_d[:, v0:v1])
                nc.tensor.matmul(
                    out=lg_ps[:, :vn],
                    lhsT=pos_ones[:, :NB],
                    rhs=bias_sl[:, :vn],
                    start=False,
                    stop=True,
                )
                u_sb = upool.tile([NB, VSLAB], F32, tag="u_sb")
                nc.scalar.activation(
                    out=u_sb[:, :vn], in_=lg_ps[:, :vn], func=ACTF.Exp,
                    accum_out=sg_cols[:, s : s + 1],
                )
                nc.sync.dma_start(out=u_out[:, v0:v1], in_=u_sb[:, :vn])
            sg_sb = small.tile([NB, 1], F32, tag="sg_sb")
            nc.vector.tensor_reduce(out=sg_sb[:], in_=sg_cols[:], axis=AX.X, op=ALU.add)
            nc.sync.dma_start(out=sg_out[:], in_=sg_sb[:])

    return nc


_NC_CACHE = None


def _get_nc():
    global _NC_CACHE
    if _NC_CACHE is None:
        nc = bacc.Bacc("TRN2", target_bir_lowering=False, debug=False, num_devices=NCORES)
        _build(nc)
        nc.compile()
        _NC_CACHE = nc
    return _NC_CACHE


def _prep_shared(inputs):
    W = np.asarray(inputs["W_out"], np.float32)
    wt_full = np.empty((KDIM, NCORES * VSH), np.float32)
    wt_full[:, :VOCAB] = W.T
    wt_full[:, VOCAB:] = 0.0
    h = np.concatenate(
        [np.asarray(inputs["encsumm"], np.float32), np.asarray(inputs["enc"], np.float32)],
        axis=1,
    )
    ht = np.ascontiguousarray(h.T)
    bias_full = np.full((1, NCORES * VSH), -30000.0, np.float32)
    with np.errstate(divide="ignore"):
        bias_full[0, :VOCAB] = np.asarray(inputs["b_out"], np.float32) + np.log(
            np.asarray(inputs["unktok_mask"], np.float32)
        )
        logmask_full = np.log(
            np.asarray(inputs["memmask"], np.float32).reshape(B, J)
        ).astype(np.float32)
    return wt_full, ht, bias_full, logmask_full, h


def _prep_core(inputs, c, wt_full, bias_full, logmask_full, ht):
    r0 = c * B_LOC
    mem_enc = np.ascontiguousarray(
        np.asarray(inputs["memencs"], np.float32)
        .reshape(B, J, DIM)[r0 : r0 + B_LOC]
        .reshape(B_LOC, NJT, 128, DIM)
    )
    mem_emb = np.ascontiguousarray(
        np.asarray(inputs["memembsumm"], np.float32)
        .reshape(B, J, EMBDIM)[r0 : r0 + B_LOC]
        .reshape(B_LOC, NJT, 128, DIM)
    )
    q = np.ascontiguousarray(
        np.concatenate(
            [
                np.asarray(inputs["enc"], np.float32)[r0 : r0 + B_LOC],
                np.asarray(inputs["embsumm"], np.float32)[r0 : r0 + B_LOC],
            ],
            axis=1,
        ).reshape(1, -1)
    )
    lm = logmask_full[r0 : r0 + B_LOC].reshape(B_LOC, NJT, 128)
    logmask = np.ascontiguousarray(lm.transpose(2, 0, 1).reshape(128, B_LOC * NJT))
    return {
        "mem_enc": mem_enc,
        "mem_emb": mem_emb,
        "q": q,
        "logmask": logmask,
        "wt": np.ascontiguousarray(wt_full[:, c * VSH : (c + 1) * VSH]),
        "ht": ht,
        "bias_gen": np.ascontiguousarray(bias_full[:, c * VSH : (c + 1) * VSH]),
    }


def _combine(inputs, core_outs, h):
    W_mix = np.asarray(inputs["W_mix"], np.float32)
    b_mix = np.asarray(inputs["b_mix"], np.float32)
    mixl = h @ W_mix.T + b_mix
    mixl -= mixl.max(axis=1, keepdims=True)
    mixe = np.exp(mixl)
    mix = mixe / mixe.sum(axis=1, keepdims=True)

    S = np.zeros((B, 1), np.float32)
    for c in range(NCORES):
        S += core_outs[c]["sg_out"]
    probs = np.empty((B, VOCAB), np.float32)
    coef_gen = (mix[:, 0:1] / S).astype(np.float32)
    for c in range(NCORES):
        v0 = c * VSH
        v1 = min(v0 + VSH, VOCAB)
        probs[:, v0:v1] = coef_gen * core_outs[c]["u_out"][:, : v1 - v0]

    memids = np.asarray(inputs["memids"]).reshape(B, J).astype(np.int64)
    mem_enc_summ = np.empty((B, DIM), np.float32)
    for c in range(NCORES):
        r0 = c * B_LOC
        e_pt = core_outs[c]["e_out"]
        e = e_pt.reshape(128, B_LOC, NJT).transpose(1, 2, 0).reshape(B_LOC, J)
        s_mem = e.sum(axis=1, keepdims=True)
        coef = mix[r0 : r0 + B_LOC, 1:2] / s_mem
        scaled = (coef * e).astype(np.float32)
        flat_ids = (
            np.arange(r0, r0 + B_LOC, dtype=np.int64)[:, None] * VOCAB
            + memids[r0 : r0 + B_LOC]
        ).ravel()
        np.add.at(probs.reshape(-1), flat_ids, scaled.ravel())
        mem_enc_summ[r0 : r0 + B_LOC] = core_outs[c]["vb_out"].reshape(B_LOC, DIM) / s_mem
    return probs, mem_enc_summ


def _run(inputs, **run_kwargs):
    wt_full, ht, bias_full, logmask_full, h = _prep_shared(inputs)
    in_maps = [
        _prep_core(inputs, c, wt_full, bias_full, logmask_full, ht)
        for c in range(NCORES)
    ]
    nc = _get_nc()
    res = run_bass_kernel_spmd(nc, in_maps, core_ids=list(range(NCORES)), **run_kwargs)
    probs, mem_enc_summ = _combine(inputs, res.results, h)
    return (probs, mem_enc_summ), res


def kernel(**inputs):
    out, _ = _run(inputs)
    return out
